# revision 1
# baseline (speedup 1.0000x reference)
"""DeepSeekV2-style single-token decode attention on 8 Trainium2 NeuronCores.

Strategy (all host-side prep is layout/sharding only; all FLOPs run on device):
  - Batch-shard attention: 8 sequences per core. Sequences are sorted by
    length and dealt round-robin so the per-slot static chunk budgets
    (baked into the single SPMD program) hug the actual seq_lens.
  - Tensor-parallel projections: w_qkv^T column-slice and w_o^T row-slice
    per core; AllToAll collectives reshard between TP and batch sharding
    (core-index-free), ReduceScatter(add) produces the final output shard.
  - K cache plane is host-transposed per sequence to [d, s] so QK^T runs
    as natural matmuls; V plane gets a ones-column appended so the softmax
    denominator accumulates in the same PSUM tile as P@V.
  - Softmax masking is fused into the ACT exp via the per-partition bias
    operand (scores kept in [s, h] layout), using host-built mask columns.
"""

import ml_dtypes
import numpy as np

import concourse.bass as bass
import concourse.tile as tile
from concourse import bacc, mybir
from concourse.bass_utils import run_bass_kernel_spmd

F32 = mybir.dt.float32
EXP = mybir.ActivationFunctionType.Exp

H, D = 32, 128
HID = 5120
Q_SIZE = H * D            # 4096
KV_SIZE = 576
QKV_OUT = Q_SIZE + KV_SIZE  # 4672
B, S_MAX = 64, 8192
SCALE = float(D) ** -0.5
NC = 8                    # cores
BPC = B // NC             # 8 seqs per core
OSL = QKV_OUT // NC       # 584 qkv output slice per core
QSL = Q_SIZE // NC        # 512 w_o contraction slice per core
NEG = -30000.0
CH_PIECE = 16             # kv chunks (of 128 positions) per DMA piece
GROUP = 4                 # score chunks per psum/exp group

_BUILD_CACHE = {}
PROFILE = False       # set True to capture an NTFF trace on the next kernel() call
LAST_RES = None       # BassKernelResults of the last run (for test harnesses)
DEBUG = False         # add intermediate-dump outputs to the build
REPEAT = 1            # run the whole body N times inside one NEFF (timing)
KV_BF16 = False       # ship K^T / V / q / probs in bf16 (halves KV traffic)
W_BF16 = False        # ship projection weights + activations in bf16
KV_F16 = False        # same but float16 (better mantissa, same traffic)
W_F16 = False


def _build(budgets, nontriv_key):
    """Build + compile the single SPMD program for the given per-slot chunk
    budgets and per-(slot, group) non-trivial-mask flags."""
    nontriv = set(nontriv_key)  # cache key: (budgets, nontriv, DEBUG, REPEAT, dtypes)
    KV_DT = mybir.dt.bfloat16 if KV_BF16 else (
        mybir.dt.float16 if KV_F16 else F32)
    W_DT = mybir.dt.bfloat16 if W_BF16 else (
        mybir.dt.float16 if W_F16 else F32)
    nc = bacc.Bacc("TRN2", target_bir_lowering=False, debug=False, num_devices=NC)

    hT = nc.dram_tensor("hT", [HID, B], W_DT, kind="ExternalInput").ap()
    wq = nc.dram_tensor("wq", [HID, OSL], W_DT, kind="ExternalInput").ap()
    wo = nc.dram_tensor("wo", [QSL, HID], W_DT, kind="ExternalInput").ap()
    ntb = nc.dram_tensor("ntb", [1, BPC], F32, kind="ExternalInput").ap()
    kts, vgs, mks = [], [], []
    for j in range(BPC):
        s_j = budgets[j] * 128
        kts.append(nc.dram_tensor(f"kt{j}", [D, s_j], KV_DT, kind="ExternalInput").ap())
        vgs.append(nc.dram_tensor(f"vg{j}", [s_j, D + 1], KV_DT, kind="ExternalInput").ap())
        mks.append(nc.dram_tensor(f"mk{j}", [128, budgets[j]], F32, kind="ExternalInput").ap())
    outp = nc.dram_tensor("outp", [BPC, HID], F32, kind="ExternalOutput").ap()

    a1_in = nc.dram_tensor("a1_in", [QKV_OUT, BPC], F32).ap()
    a1_out = nc.dram_tensor("a1_out", [QKV_OUT, BPC], F32).ap()
    at_in = nc.dram_tensor("at_in", [Q_SIZE, BPC], F32).ap()
    at_out = nc.dram_tensor("at_out", [Q_SIZE, BPC], F32).ap()
    po_b = nc.dram_tensor("po_b", [B, HID], F32).ap()
    rs_o = nc.dram_tensor("rs_o", [BPC, HID], F32).ap()

    groups = [list(range(NC))]
    HC = HID // 128               # 40 hidden chunks
    OCS = [128, 128, 128, 128, OSL - 512]   # qkv slice partition chunks

    with tile.TileContext(nc) as tc:
        with (
            tc.tile_pool(name="wts", bufs=2) as wts_pool,
            tc.tile_pool(name="acts", bufs=1) as acts_pool,
            tc.tile_pool(name="kv", bufs=3) as kv_pool,
            tc.tile_pool(name="probs", bufs=4) as probs_pool,
            tc.tile_pool(name="small", bufs=2) as small_pool,
            tc.tile_pool(name="psA", bufs=2, space="PSUM") as psA,
            tc.tile_pool(name="psSc", bufs=3, space="PSUM") as psSc,
            tc.tile_pool(name="psAt", bufs=2, space="PSUM") as psAt,
        ):
          def body():
            # ---------- Phase 1: qkv projection (TP column slice) ----------
            hT_t = acts_pool.tile([128, HC, B], W_DT)
            nc.sync.dma_start(hT_t[:, :, :], hT.rearrange("(c p) b -> p c b", p=128))

            a1_view = a1_in.rearrange("(d o) j -> o d j", d=NC)  # [OSL, NC, BPC]
            for oc in range(5):
                osz = OCS[oc]
                wq_t = wts_pool.tile([128, HC, 128], W_DT, tag="wq")
                nc.sync.dma_start(
                    wq_t[:, :, :osz],
                    wq.rearrange("(c p) o -> p c o", p=128)[
                        :, :, oc * 128:oc * 128 + osz],
                )
                ps_q = psA.tile([128, B], F32, tag="mm")
                for hc in range(HC):
                    nc.tensor.matmul(
                        ps_q[:osz, :],
                        lhsT=wq_t[:, hc, :osz],
                        rhs=hT_t[:, hc, :],
                        start=(hc == 0), stop=(hc == HC - 1),
                    )
                q_sb = small_pool.tile([128, B], F32, tag="qsb")
                nc.vector.tensor_copy(q_sb[:osz, :], ps_q[:osz, :])
                nc.sync.dma_start(
                    a1_view[oc * 128:oc * 128 + osz, :, :],
                    q_sb[:osz, :].rearrange("o (d j) -> o d j", d=NC),
                )

            nc.gpsimd.collective_compute(
                "AllToAll", mybir.AluOpType.bypass, replica_groups=groups,
                ins=[a1_in[:, :]], outs=[a1_out[:, :]],
            )

            # ---------- Phase 2: per-core q / kv_new ----------
            dma_kv = nc.gpsimd if KV_DT != F32 else nc.sync  # SWDGE casts
            qt_t = acts_pool.tile([128, H, BPC], KV_DT)   # q^T: [d, head, slot]
            dma_kv.dma_start(
                qt_t[:, :, :],
                a1_out[0:Q_SIZE, :].rearrange("(h p) j -> p h j", p=128),
            )
            kvnT_t = acts_pool.tile([128, BPC], KV_DT)    # kv_new^T: [d, slot]
            dma_kv.dma_start(kvnT_t[:, :], a1_out[Q_SIZE:Q_SIZE + D, :])
            kvnr_t = acts_pool.tile([1, BPC * (D + 1)], KV_DT)  # kv_new rows + ones
            for j in range(BPC):
                dma_kv.dma_start(
                    kvnr_t[0:1, j * (D + 1):j * (D + 1) + D],
                    a1_out[Q_SIZE:Q_SIZE + D, j:j + 1].rearrange("q j -> j q"),
                )
                nc.vector.memset(
                    kvnr_t[0:1, j * (D + 1) + D:(j + 1) * (D + 1)], 1.0)
            ntb_t = acts_pool.tile([1, BPC], F32)
            nc.sync.dma_start(ntb_t[:, :], ntb[:, :])

            at_all_t = acts_pool.tile([H, D, BPC], F32)  # attn for all 8 slots

            # ---------- Phase 3: attention, one slot (sequence) at a time ----------
            for j in range(BPC):
                bj = budgets[j]
                mk_t = small_pool.tile([128, bj], F32, tag="mk")
                nc.sync.dma_start(mk_t[:, :], mks[j][:, :])
                attn_ps = psAt.tile([H, D + 1], F32, tag="at")
                qt_j = qt_t[:, :, j]
                n_mm = 0
                for p0 in range(0, bj, CH_PIECE):
                    pc = min(CH_PIECE, bj - p0)
                    kt_t = kv_pool.tile([128, CH_PIECE * 128], KV_DT, tag="kt")
                    nc.sync.dma_start(
                        kt_t[:, :pc * 128], kts[j][:, p0 * 128:(p0 + pc) * 128])
                    vg_t = kv_pool.tile([128, CH_PIECE, D + 1], KV_DT, tag="vg")
                    nc.sync.dma_start(
                        vg_t[:, :pc, :],
                        vgs[j][p0 * 128:(p0 + pc) * 128, :].rearrange(
                            "(c p) e -> p c e", p=128),
                    )
                    for g0 in range(0, pc, GROUP):
                        gs = min(GROUP, pc - g0)
                        ps_sc = psSc.tile([128, GROUP * H], F32, tag="sc")
                        for k in range(gs):
                            nc.tensor.matmul(
                                ps_sc[:, k * H:(k + 1) * H],
                                lhsT=kt_t[:, (g0 + k) * 128:(g0 + k + 1) * 128],
                                rhs=qt_j,
                                start=True, stop=True,
                            )
                        pt = probs_pool.tile([128, GROUP * H], KV_DT, tag="pt")
                        grp_id = (p0 + g0) // GROUP
                        if (j, grp_id) in nontriv:
                            for k in range(gs):
                                ch = p0 + g0 + k
                                nc.scalar.activation(
                                    pt[:, k * H:(k + 1) * H],
                                    ps_sc[:, k * H:(k + 1) * H],
                                    EXP, bias=mk_t[:, ch:ch + 1], scale=SCALE,
                                )
                        else:
                            nc.scalar.activation(
                                pt[:, :gs * H], ps_sc[:, :gs * H], EXP, scale=SCALE)
                        for k in range(gs):
                            nc.tensor.matmul(
                                attn_ps[:, :],
                                lhsT=pt[:, k * H:(k + 1) * H],
                                rhs=vg_t[:, g0 + k, :],
                                start=(n_mm == 0), stop=False,
                            )
                            n_mm += 1
                # new-token term (Kc=1 outer product adds p_new to attn and denom)
                ps_nt = psSc.tile([1, H], F32, tag="sc")
                nc.tensor.matmul(
                    ps_nt[:, :], lhsT=kvnT_t[:, j:j + 1], rhs=qt_j,
                    start=True, stop=True)
                pn_t = small_pool.tile([1, H], KV_DT, tag="pn")
                nc.scalar.activation(
                    pn_t[:, :], ps_nt[:, :], EXP,
                    bias=ntb_t[0:1, j:j + 1], scale=SCALE)
                nc.tensor.matmul(
                    attn_ps[:, :], lhsT=pn_t[0:1, :],
                    rhs=kvnr_t[0:1, j * (D + 1):(j + 1) * (D + 1)],
                    start=False, stop=True)

                rc_t = small_pool.tile([H, 1], F32, tag="rc")
                nc.vector.reciprocal(rc_t[:, :], attn_ps[:, D:D + 1])
                nc.vector.tensor_scalar_mul(
                    at_all_t[:, :, j], attn_ps[:, 0:D], rc_t[:, :])

            # at_in[(h*128+dd), j]; a2a block d = rows [d*512, (d+1)*512)
            nc.sync.dma_start(
                at_in.rearrange("(h x) j -> h x j", h=H), at_all_t[:, :, :])

            nc.gpsimd.collective_compute(
                "AllToAll", mybir.AluOpType.bypass, replica_groups=groups,
                ins=[at_in[:, :]], outs=[at_out[:, :]],
            )

            # ---------- Phase 4: output projection (TP row slice) ----------
            dma_w = nc.gpsimd if W_DT != F32 else nc.sync
            ao_t = acts_pool.tile([128, QSL // 128, NC, BPC], W_DT)
            ao_view = at_out.rearrange("(i c p) j -> p c i j", i=NC, p=128)
            for qc in range(QSL // 128):
                dma_w.dma_start(ao_t[:, qc, :, :], ao_view[:, qc, :, :])
            WO_PIECE = 1024       # output cols per wo tile piece (divisible by 512)
            for w in range(HID // WO_PIECE):
                wo_t = wts_pool.tile([128, QSL // 128, WO_PIECE], W_DT, tag="wo")
                nc.sync.dma_start(
                    wo_t[:, :, :],
                    wo.rearrange("(c p) o -> p c o", p=128)[
                        :, :, w * WO_PIECE:(w + 1) * WO_PIECE],
                )
                for nb in range(WO_PIECE // 512):
                    ps_o = psA.tile([B, 512], F32, tag="mm")
                    for qc in range(QSL // 128):
                        nc.tensor.matmul(
                            ps_o[:, :],
                            lhsT=ao_t[:, qc, :, :],
                            rhs=wo_t[:, qc, nb * 512:(nb + 1) * 512],
                            start=(qc == 0), stop=(qc == QSL // 128 - 1),
                        )
                    po_sb = small_pool.tile([B, 512], F32, tag="pos")
                    nc.vector.tensor_copy(po_sb[:, :], ps_o[:, :])
                    nc.sync.dma_start(
                        po_b[:, w * WO_PIECE + nb * 512:w * WO_PIECE + (nb + 1) * 512],
                        po_sb[:, :])

            nc.gpsimd.collective_compute(
                "ReduceScatter", mybir.AluOpType.add, replica_groups=groups,
                ins=[po_b[:, :]], outs=[rs_o[:, :]],
            )
            nc.sync.dma_start(outp[:, :], rs_o[:, :])

          for _rep in range(REPEAT):
              body()
          if DEBUG:
              for nm, s_ap in [("dbg_a1", a1_out), ("dbg_ati", at_in),
                               ("dbg_ato", at_out), ("dbg_po", po_b)]:
                  dst = nc.dram_tensor(nm, list(s_ap.shape), F32,
                                       kind="ExternalOutput").ap()
                  nc.sync.dma_start(dst[:, :], s_ap[:, :])

    nc.compile()
    return nc


def _prepare(hidden_states, positions, kv_cache, slot_mapping, seq_lens, w_qkv, w_o):
    """Host-side sharding/layout prep. Returns (nc, in_maps, col_seq)."""
    hidden_states = np.asarray(hidden_states, dtype=np.float32)
    kv_cache = np.asarray(kv_cache, dtype=np.float32)
    slot_mapping = np.asarray(slot_mapping)
    seq_lens = np.asarray(seq_lens)
    w_qkv = np.asarray(w_qkv, dtype=np.float32)
    w_o = np.asarray(w_o, dtype=np.float32)

    sl = seq_lens.astype(np.int64)
    sm = slot_mapping.astype(np.int64)

    # sort by length desc, deal round-robin: core c slot j <- rank 8j + c
    order = np.argsort(-sl, kind="stable")
    seq_of = np.empty((NC, BPC), dtype=np.int64)
    for j in range(BPC):
        for c in range(NC):
            seq_of[c, j] = order[NC * j + c]
    col_seq = seq_of.reshape(NC * BPC)  # global column order (c, j)

    budgets = []
    for j in range(BPC):
        max_l = int(sl[seq_of[:, j]].max())
        budgets.append(max(1, -(-max_l // 128)))
    budgets = tuple(budgets)

    # masks + non-trivial group flags
    masks = [np.zeros((NC, 128, budgets[j]), dtype=np.float32) for j in range(BPC)]
    nontriv = set()
    for c in range(NC):
        for j in range(BPC):
            b = seq_of[c, j]
            L, slot = int(sl[b]), int(sm[b])
            n = budgets[j] * 128
            m = np.zeros(n, dtype=np.float32)
            if L < n:
                m[L:] = NEG
            if slot < n:
                m[slot] = NEG
            mc = m.reshape(budgets[j], 128)
            masks[j][c] = mc.T
            for ch in np.nonzero(mc.any(axis=1))[0]:
                nontriv.add((j, int(ch) // GROUP))
    nontriv_key = tuple(sorted(nontriv))

    key = (budgets, nontriv_key, DEBUG, REPEAT, KV_BF16, W_BF16, KV_F16, W_F16)
    if key not in _BUILD_CACHE:
        _BUILD_CACHE[key] = _build(budgets, nontriv_key)
    nc = _BUILD_CACHE[key]

    w_np = ml_dtypes.bfloat16 if W_BF16 else (
        np.float16 if W_F16 else np.float32)
    kv_np = ml_dtypes.bfloat16 if KV_BF16 else (
        np.float16 if KV_F16 else np.float32)
    hT = np.ascontiguousarray(hidden_states[col_seq, 0, :].T).astype(w_np)
    wqT = np.ascontiguousarray(w_qkv.T)                             # [HID, 4672]
    woT = np.ascontiguousarray(w_o.T)                               # [4096, HID]

    in_maps = []
    for c in range(NC):
        m = {
            "hT": hT,
            "wq": np.ascontiguousarray(wqT[:, c * OSL:(c + 1) * OSL]).astype(w_np),
            "wo": np.ascontiguousarray(woT[c * QSL:(c + 1) * QSL, :]).astype(w_np),
            "ntb": np.where(sm[seq_of[c]] < sl[seq_of[c]], 0.0, NEG
                            ).astype(np.float32).reshape(1, BPC),
        }
        for j in range(BPC):
            b = seq_of[c, j]
            n = budgets[j] * 128
            m[f"kt{j}"] = np.ascontiguousarray(kv_cache[0, b, :n, :].T).astype(kv_np)
            vg = np.empty((n, D + 1), dtype=kv_np)
            vg[:, :D] = kv_cache[1, b, :n, :]
            vg[:, D] = 1.0
            m[f"vg{j}"] = vg
            m[f"mk{j}"] = np.ascontiguousarray(masks[j][c])
        in_maps.append(m)

    return nc, in_maps, col_seq


def kernel(hidden_states, positions, kv_cache, slot_mapping, seq_lens, w_qkv, w_o):
    nc, in_maps, col_seq = _prepare(
        hidden_states, positions, kv_cache, slot_mapping, seq_lens, w_qkv, w_o)
    res = run_bass_kernel_spmd(nc, in_maps, list(range(NC)), trace=PROFILE)
    global LAST_RES
    LAST_RES = res

    out = np.empty((B, 1, HID), dtype=np.float32)
    for c in range(NC):
        shard = res.results[c]["outp"]
        for j in range(BPC):
            out[col_seq[c * BPC + j], 0, :] = shard[j]
    return out

